# revision 5
# baseline (speedup 1.0000x reference)
"""Brute-force L2 1-NN on 8 TRN2 NeuronCores.

Problem: x [4096, 256], prototypes [32768, 256] -> prototypes[argmin_j ||x-p_j||^2]

Strategy (prototype-sharded SPMD, no collectives):
  - Each core owns a 4096-row shard of the prototype bank; queries replicated.
  - Scores via TensorE fp32r matmuls in [q_part, j_free] orientation with an
    augmented contraction that folds in the |p|^2 term:
      c'[q, j] = x.p - 0.5|p|^2   (argmax_j c' == argmin_j ||x-p||^2)
    K = 256 (two 128-chunks) + one aug chunk (row of ones in x^T, row of
    -0.5|p|^2 in P^T).
  - ScalarE copies each PSUM tile to SBUF; VectorE tensor_reduce(max) gives
    per-512-chunk maxes m[q, 8]; max_index recovers each chunk-max position
    (exact fp32 value match).
  - Host combines 8 cores x 8 chunks = 64 exact (value, index) candidates per
    query, rescores the near-maximal ones exactly in float64, gathers rows.

The fp32r matmul is TF32-ish (~1.5e-2 abs error on these magnitudes); the
host rescore threshold (0.25) covers it with huge margin, so the final
argmin is exact.
"""

import sys
import types

sys.path.insert(0, "/opt/trn_rl_repo")


def _install_ntff_hook():
    try:
        from trn_agent_boot.trn_boot import _ntff_profile_via_ctypes
    except ImportError:
        return
    try:
        hook = _ntff_profile_via_ctypes("/opt/axon/libaxon_pjrt.so")
    except OSError:
        return
    mod = types.ModuleType("antenv.axon_hooks")
    _h = [hook]
    mod.get_axon_ntff_profile_hook = lambda: _h[0]
    mod.set_axon_ntff_profile_hook = lambda h: _h.__setitem__(0, h)
    sys.modules["antenv.axon_hooks"] = mod
    import antenv

    antenv.axon_hooks = mod


_install_ntff_hook()

import numpy as np
import concourse.bass as bass
import concourse.mybir as mybir
import concourse.tile as tile
from concourse import bacc
from concourse.bass_utils import run_bass_kernel_spmd

B, N, D = 4096, 32768, 256
NCORES = 8
NLOC = N // NCORES  # 4096 prototypes per core
QT = 128  # queries per tile
NQT = B // QT  # 32 query tiles
JC = 512  # j-chunk width (one psum bank)
NJC = NLOC // JC  # 8 chunks per core
HF = NJC // 2  # chunks per psum half-tile


def build(nqt=NQT, njc=NJC):
    """Build the per-core Bass graph. nqt/njc shrinkable for simulation."""
    f32 = mybir.dt.float32
    f32r = mybir.dt.float32r
    u32 = mybir.dt.uint32
    nloc = njc * JC
    b = nqt * QT
    hf = max(1, njc // 2)

    nc = bacc.Bacc("TRN2", target_bir_lowering=False, debug=False, num_devices=NCORES)
    xT_d = nc.dram_tensor("xT", [2, 128, b], f32r, kind="ExternalInput").ap()
    pT_d = nc.dram_tensor("pT", [2, 128, nloc], f32r, kind="ExternalInput").ap()
    xa_d = nc.dram_tensor("xa", [8, b], f32r, kind="ExternalInput").ap()
    pa_d = nc.dram_tensor("pa", [8, nloc], f32r, kind="ExternalInput").ap()
    m_out = nc.dram_tensor("m", [nqt, QT, njc], f32, kind="ExternalOutput").ap()
    idx_out = nc.dram_tensor("idx", [nqt, QT, njc, 8], u32, kind="ExternalOutput").ap()

    with tile.TileContext(nc) as tc:
        with (
            tc.tile_pool(name="persist", bufs=1) as pp,
            tc.tile_pool(name="dbuf", bufs=3) as dbuf,
            tc.tile_pool(name="small", bufs=4) as sp,
            tc.tile_pool(name="ps", bufs=2, space="PSUM") as ps,
        ):
            xT_sb = pp.tile([128, 2, b], f32r)
            pT_sb = pp.tile([128, 2, nloc], f32r)
            xa_sb = pp.tile([8, b], f32r)
            pa_sb = pp.tile([8, nloc], f32r)
            for k in range(2):
                nc.sync.dma_start(xT_sb[:, k, :], xT_d[k])
                nc.sync.dma_start(pT_sb[:, k, :], pT_d[k])
            nc.sync.dma_start(xa_sb[:], xa_d)
            nc.sync.dma_start(pa_sb[:], pa_d)

            for qt in range(nqt):
                qs = bass.ts(qt, QT)
                m_sb = sp.tile([QT, 8], f32, tag="m")
                if njc < 8:
                    nc.vector.memset(m_sb[:], -3.0e38)
                d_sb = dbuf.tile([QT, njc, JC], f32, tag="d", name=f"d{qt}")
                for h in range(njc // hf):
                    psum_h = ps.tile([QT, hf, JC], f32, tag="psb", name=f"ps{qt}_{h}")
                    for jc in range(hf):
                        for k in range(3):
                            lhs = (
                                xa_sb[:, qs]
                                if k == 2
                                else xT_sb[:, k, qs]
                            )
                            rhs = (
                                pa_sb[:, bass.ts(h * hf + jc, JC)]
                                if k == 2
                                else pT_sb[:, k, bass.ts(h * hf + jc, JC)]
                            )
                            nc.tensor.matmul(
                                psum_h[:, jc, :],
                                lhs,
                                rhs,
                                start=(k == 0),
                                stop=(k == 2),
                            )
                    # ScalarE: PSUM -> SBUF copy of this half
                    nc.scalar.copy(
                        d_sb[:, h * hf : (h + 1) * hf, :], psum_h[:]
                    )
                    # VectorE: per-chunk max over the half (read from PSUM)
                    nc.vector.tensor_reduce(
                        m_sb[:, h * hf : (h + 1) * hf],
                        psum_h[:],
                        axis=mybir.AxisListType.X,
                        op=mybir.AluOpType.max,
                    )
                idx_sb = sp.tile([QT, njc, 8], u32, tag="idx")
                for jc in range(njc):
                    nc.vector.max_index(idx_sb[:, jc, :], m_sb[:], d_sb[:, jc, :])
                nc.sync.dma_start(m_out[qt], m_sb[:, 0:njc])
                nc.sync.dma_start(idx_out[qt], idx_sb[:])
    nc.compile()
    return nc


def _prep_inputs(x, prototypes):
    """Host-side shard prep: transposes, aug rows, sharding."""
    xT = np.ascontiguousarray(x.T).reshape(2, 128, B)
    xa = np.zeros((8, B), dtype=np.float32)
    xa[0] = 1.0
    in_maps = []
    for c in range(NCORES):
        P = prototypes[c * NLOC : (c + 1) * NLOC]
        pT = np.ascontiguousarray(P.T).reshape(2, 128, NLOC)
        pa = np.zeros((8, NLOC), dtype=np.float32)
        pa[0] = -0.5 * np.einsum("jd,jd->j", P, P)
        in_maps.append({"xT": xT, "pT": pT, "xa": xa, "pa": pa})
    return in_maps


_NC_CACHE = {}


def kernel(x: np.ndarray, prototypes: np.ndarray) -> np.ndarray:
    x = np.asarray(x, dtype=np.float32)
    prototypes = np.asarray(prototypes, dtype=np.float32)
    assert x.shape == (B, D) and prototypes.shape == (N, D)

    if "nc" not in _NC_CACHE:
        _NC_CACHE["nc"] = build()
    nc = _NC_CACHE["nc"]
    in_maps = _prep_inputs(x, prototypes)
    res = run_bass_kernel_spmd(nc, in_maps, core_ids=list(range(NCORES)))
    _NC_CACHE["last_results"] = res

    # m_all[c, q, jc]: per-chunk max of c' = x.p - 0.5|p|^2  (fp32r-accurate)
    # idx_all[c, q, jc]: position of that max within its 512-wide chunk
    m_all = np.stack([res.results[c]["m"].reshape(B, NJC) for c in range(NCORES)])
    idx_raw = np.stack(
        [res.results[c]["idx"].reshape(B, NJC, 8) for c in range(NCORES)]
    )
    idx_all = idx_raw[:, :, np.arange(NJC), np.arange(NJC)].astype(np.int64)

    core_off = (np.arange(NCORES) * NLOC)[:, None, None]
    chunk_off = (np.arange(NJC) * JC)[None, None, :]
    gidx = core_off + chunk_off + idx_all  # [8, B, 8]

    m_flat = np.transpose(m_all, (1, 0, 2)).reshape(B, NCORES * NJC)
    g_flat = np.transpose(gidx, (1, 0, 2)).reshape(B, NCORES * NJC)

    # Candidates: within DELTA of the per-query best measured c'. fp32r error
    # is <~0.02 absolute here; 0.25 is overkill-safe. Rescore exactly in
    # float64, argmax c' (ties -> lowest index, matching jnp.argmin).
    DELTA = 0.25
    best = m_flat.max(axis=1, keepdims=True)
    qs, cs = np.nonzero(m_flat >= best - DELTA)
    cand_j = g_flat[qs, cs]
    pc = prototypes[cand_j].astype(np.float64)
    xc = x[qs].astype(np.float64)
    c_exact = np.einsum("ij,ij->i", pc, xc) - 0.5 * np.einsum("ij,ij->i", pc, pc)
    order = np.lexsort((cand_j, -c_exact, qs))
    qs_o = qs[order]
    first = np.unique(qs_o, return_index=True)[1]
    out_idx = np.empty(B, dtype=np.int64)
    out_idx[qs_o[first]] = cand_j[order][first]

    return prototypes[out_idx]


if __name__ == "__main__":
    rng = np.random.default_rng(0)
    x = rng.standard_normal((B, D), dtype=np.float32)
    p = rng.standard_normal((N, D), dtype=np.float32)
    out = kernel(x, p)
    print("out", out.shape, out.dtype)
